# revision 15
# baseline (speedup 1.0000x reference)
"""Causal self-attention (B=4, T=2048, C=1024, H=16, D=64) on 8 TRN2 cores.

Sharding: core c handles batch b = c//2 and head-half hh = c%2 (8 heads).
Each core computes the qkv projection for its heads, causal attention, and
a partial output projection (its heads' rows of W_proj). Host sums the two
partials per batch and adds b_proj.

Schedule (single software-pipelined stream; all matmuls bf16, fp32 PSUM):
  - Attention blocks (tj, hp) run tj-DESCENDING (3,2,1,0), hp 0..3, so the
    largest exp workloads come first (when qk/v projection filler work is
    plentiful) and the tail block is tiny.
  - Prefix: k-proj for hp=0, all V projection (va tiles), q-proj(hp0,tj3),
    chasing the DMA stream (wqk-ch4 + xt quarter-columns first).
  - A global filler queue holds the remaining qk-proj units and the output
    projection tiles; between attention s-tiles the scheduler pops fillers
    by a deficit model (ScalarE exp time minus PE S/PV time per s-tile) to
    keep the PE busy while exp runs.
  - DMA instruction count is minimized (the Sync engine issues descriptors
    serially at ~600ns each): xt is one [128,8,2048] tile filled by 4
    column-quarter DMAs, wqk one DMA per 128-channel group, wv/wp one DMA
    each, and the va ones-column is written by GpSimd copies, not DMA.
  - Causal diagonal handled by narrowing S/exp/PV to the valid column
    range plus a [128,2,128] triangular mask multiply on GpSimd.
  - Normalization: row sums ride in PV output row D (ones column in va);
    one SBUF restriding DMA to a [128,8] scatter layout, DVE reciprocal,
    one bounce back to a [1,1024] row, broadcast across the 64 partitions
    with a K=1 matmul, applied in the DVE mult into ysb.
"""

from contextlib import ExitStack

import ml_dtypes
import numpy as np

import concourse.bass as bass
import concourse.tile as tile
from concourse import bacc, mybir
from concourse.bass_utils import run_bass_kernel_spmd

F32 = mybir.dt.float32
DT = mybir.dt.bfloat16
NPDT = ml_dtypes.bfloat16
EXP = mybir.ActivationFunctionType.Exp

T = 2048        # tokens per core (one batch element)
C = 1024        # embed dim
H = 8           # local heads per core
D = 64          # head dim
P = 128
CT = C // P     # 8 contraction tiles over embed dim
QC = H * D      # 512 q/k/v channels per core
TJN = T // 512  # 4 t-tiles (free dim) for attention
SIN = T // P    # 16 s-tiles

TRACE = False   # set by test.py for profiling runs


def build_program():
    nc = bacc.Bacc("TRN2", target_bir_lowering=False, debug=False)
    xT = nc.dram_tensor("xT", [C, T], DT, kind="ExternalInput").ap()
    wqk = nc.dram_tensor("wqk", [C, 2 * QC], DT, kind="ExternalInput").ap()
    bqk = nc.dram_tensor("bqk", [2 * QC], F32, kind="ExternalInput").ap()
    wv = nc.dram_tensor("wv", [C, QC], DT, kind="ExternalInput").ap()
    wp = nc.dram_tensor("wp", [QC, C], DT, kind="ExternalInput").ap()
    trimask = nc.dram_tensor("trimask", [P, P], DT, kind="ExternalInput").ap()
    ones_in = nc.dram_tensor("ones", [P, P], DT, kind="ExternalInput").ap()
    out = nc.dram_tensor("out", [T, C], DT, kind="ExternalOutput").ap()

    with tile.TileContext(nc) as tc, ExitStack() as persist:
        p_small = persist.enter_context(tc.tile_pool(name="small", bufs=1))
        p_qkt = persist.enter_context(tc.tile_pool(name="qkt", bufs=1))
        p_va = persist.enter_context(tc.tile_pool(name="va", bufs=1))
        qkt = [p_qkt.tile([P, T], DT, tag=f"qkt{i}", name=f"qkt{i}") for i in range(CT)]
        va = [p_va.tile([P, H, D + 1], DT, tag=f"va{i}", name=f"va{i}") for i in range(SIN)]

        with ExitStack() as ph:
            p_xt = ph.enter_context(tc.tile_pool(name="xt", bufs=1))
            p_wqk = ph.enter_context(tc.tile_pool(name="wqk", bufs=8))
            p_wv = ph.enter_context(tc.tile_pool(name="wv", bufs=1))
            xt = p_xt.tile([P, CT, T], DT, tag="xt", name="xt")
            wv_sb = p_wv.tile([P, CT, QC], DT, tag="wv", name="wv")

            p_ysb = ph.enter_context(tc.tile_pool(name="ysb", bufs=1))
            ysb = [p_ysb.tile([P, T], DT, tag=f"ysb{i}", name=f"ysb{i}")
                   for i in range(QC // P)]
            p_wp = ph.enter_context(tc.tile_pool(name="wp", bufs=1))
            wpt = p_wp.tile([P, QC // P, C], DT, tag="wp", name="wp")
            p_pt = ph.enter_context(tc.tile_pool(name="pt", bufs=4))
            p_scat = ph.enter_context(tc.tile_pool(name="scat", bufs=4))
            p_rcpr = ph.enter_context(tc.tile_pool(name="rcpr", bufs=5))
            p_yun = ph.enter_context(tc.tile_pool(name="yun", bufs=5))
            p_yn = ph.enter_context(tc.tile_pool(name="yn", bufs=3))
            p_rb = ph.enter_context(tc.tile_pool(name="rb", bufs=3))
            p_o = ph.enter_context(tc.tile_pool(name="o", bufs=2))
            ps_s = ph.enter_context(tc.tile_pool(name="ps_s", bufs=2, space="PSUM"))
            ps_y = ph.enter_context(tc.tile_pool(name="ps_y", bufs=2, space="PSUM"))
            # shared by v-proj, qk-proj, normalization R, and proj outputs
            ps_r = ph.enter_context(tc.tile_pool(name="ps_r", bufs=2, space="PSUM"))

            # ---------- DMA emission (defines Sync-queue order) ----------
            qk_w = [None] * (2 * QC // P)

            def dma_qk_w(ch):
                t = p_wqk.tile([P, CT, P], DT, tag="wqk", name=f"wqk{ch}")
                nc.sync.dma_start(
                    t, wqk.rearrange("(j p) c -> p j c", p=P)[:, :, ch * P:(ch + 1) * P])
                qk_w[ch] = t

            xTr = xT.rearrange("(j p) t -> p j t", p=P)
            dma_qk_w(4)                      # k-weights for hp=0 first
            # first x quarter in two halves so qkproj(4,0) starts sooner
            nc.sync.dma_start(xt[:, 0:4, 0:512], xTr[:, 0:4, 0:512])
            nc.sync.dma_start(xt[:, 4:8, 0:512], xTr[:, 4:8, 0:512])
            nc.sync.dma_start(wv_sb, wv.rearrange("(j p) c -> p j c", p=P))

            bqk_sb = p_small.tile([P, CT], F32, tag="bqk")
            nc.sync.dma_start(bqk_sb, bqk.rearrange("(j p) -> p j", p=P))
            ones_sb = p_small.tile([P, P], DT, tag="ones_sb")
            nc.sync.dma_start(ones_sb, ones_in)
            tri_sb = p_small.tile([P, P], DT, tag="tri")
            nc.sync.dma_start(tri_sb, trimask)
            onesf_sb = p_small.tile([P, D], mybir.dt.float32r, tag="onesf_sb")
            nc.vector.tensor_copy(onesf_sb, ones_sb[:, 0:D])

            for tq in range(1, 4):           # remaining x quarters
                nc.sync.dma_start(xt[:, :, tq * 512:(tq + 1) * 512],
                                  xTr[:, :, tq * 512:(tq + 1) * 512])
            dma_qk_w(0)
            for ch in (5, 1, 6, 2, 7, 3):
                dma_qk_w(ch)
            nc.sync.dma_start(wpt, wp.rearrange("(i p) c -> p i c", p=P))

            # ---------- work units ----------
            def qkproj(ch, tjc):
                wt = qk_w[ch]
                pq = ps_r.tile([P, 512], F32, tag="ps_r", name="pq")
                for j in range(CT):
                    nc.tensor.matmul(
                        pq, lhsT=wt[:, j, :],
                        rhs=xt[:, j, tjc * 512:(tjc + 1) * 512],
                        start=(j == 0), stop=(j == CT - 1))
                nc.vector.tensor_scalar_add(
                    out=qkt[ch][:, tjc * 512:(tjc + 1) * 512],
                    in0=pq, scalar1=bqk_sb[:, ch:ch + 1])

            def vproj(g):
                # v-bias omitted: softmax weights sum to 1, so b_v passes
                # through attention unchanged; host folds b_v @ W_proj into
                # the output bias instead.
                for tt in (2 * g, 2 * g + 1):
                    pv = ps_r.tile([P, QC], F32, tag="ps_r", name="pv")
                    for j in range(CT):
                        nc.tensor.matmul(
                            pv, lhsT=xt[:, j, tt * P:(tt + 1) * P],
                            rhs=wv_sb[:, j, :], start=(j == 0),
                            stop=(j == CT - 1))
                    nc.vector.tensor_copy(out=va[tt][:, :, 0:D], in_=pv)
                    nc.gpsimd.tensor_copy(out=va[tt][:, :, D:D + 1],
                                          in_=ones_sb[:, 0:H][:, :, None])

            def proj_tile(tt):
                ot = p_o.tile([P, C], DT, tag="o", name="ot")
                for co in range(C // 512):
                    po = ps_r.tile([P, 512], F32, tag="ps_r", name="po")
                    for i in range(QC // P):
                        nc.tensor.matmul(
                            po, lhsT=ysb[i][:, tt * P:(tt + 1) * P],
                            rhs=wpt[:, i, co * 512:(co + 1) * 512],
                            start=(i == 0), stop=(i == QC // P - 1))
                    nc.vector.tensor_copy(ot[:, co * 512:(co + 1) * 512], po)
                nc.sync.dma_start(out[tt * P:(tt + 1) * P, :], ot)

            def norm_batch(hp, tj, rcp_row, yun):
                ts = slice(tj * 512, (tj + 1) * 512)
                for head in (0, 1):
                    r = p_rb.tile([D, 512], mybir.dt.float32r, tag="rb", name="rb")
                    nc.gpsimd.partition_broadcast(
                        r, rcp_row[0:1, head * 512:(head + 1) * 512])
                    if head == 0:
                        nc.vector.tensor_mul(ysb[hp][0:D, ts],
                                             yun[0:D, 0:512], r)
                    else:
                        ynb = p_yn.tile([D, 512], DT, tag="yn", name="ynb")
                        nc.vector.tensor_mul(ynb, yun[0:D, 512:1024], r)
                        nc.sync.dma_start(ysb[hp][D:P, ts], ynb)

            # ---------- filler queue ----------
            fillq = []
            emitted = set()

            def push(cost, key, fn):
                fillq.append((cost, key, fn))

            def pop_front():
                cost, key, fn = fillq.pop(0)
                emitted.add(key)
                fn()
                return cost

            def ensure(key):
                while key not in emitted:
                    pop_front()

            QP = 1760.0   # ns per 8-MM qk-proj unit
            OP = 1760.0   # ns per out-proj row-tile (both column halves)
            for hp in range(1, 4):
                for tjc in range(TJN):
                    push(QP, ("k", hp, tjc),
                         (lambda c=4 + hp, t=tjc: qkproj(c, t)))
                push(QP, ("q", hp, 3), (lambda c=hp: qkproj(c, 3)))
            for tj in (2, 1, 0):
                for hp in range(4):
                    push(QP, ("q", hp, tj),
                         (lambda c=hp, t=tj: qkproj(c, t)))

            # ---------- prefix: hp0 k-proj + all V proj + q(0,3) ----------
            qkproj(4, 0)
            vproj(0)
            vproj(1)
            qkproj(4, 1)
            vproj(2)
            vproj(3)
            qkproj(4, 2)
            vproj(4)
            vproj(5)
            qkproj(4, 3)
            vproj(6)
            vproj(7)
            qkproj(0, 3)
            for t in range(TJN):
                emitted.add(("k", 0, t))
            emitted.add(("q", 0, 3))

            # ---------- attention blocks ----------
            pending = []
            norms_emitted = {tj: 0 for tj in range(TJN)}
            deficit = 0.0

            def emit_norm(args):
                hp, tj = args[0], args[1]
                norm_batch(*args)
                norms_emitted[tj] += 1
                if norms_emitted[tj] == 4:
                    for tt in range(4 * tj, 4 * tj + 4):
                        push(OP, ("o", tt), (lambda a=tt: proj_tile(a)))

            blocks = [(tj, hp) for tj in (3, 2, 1, 0) for hp in range(4)]
            for bi, (tj, hp) in enumerate(blocks):
                    if (tj, hp) != (3, 0):
                        ensure(("q", hp, tj))
                    if bi + 1 < len(blocks):
                        # prefetch next block's q-proj so its S matmuls
                        # don't wait on the PSUM->qkt cast latency
                        ntj, nhp = blocks[bi + 1]
                        ensure(("q", nhp, ntj))
                    qt, kt = qkt[hp], qkt[4 + hp]
                    nsi = 4 * tj + 4
                    ya = ps_y.tile([D + 1, 512], F32, tag="ps_y")
                    yb = ps_y.tile([D + 1, 512], F32, tag="ps_y")

                    def pv_step(pt, si, last):
                        o = max(si - 4 * tj, 0) * P
                        nc.tensor.matmul(
                            ya[:, o:512], lhsT=va[si][:, 2 * hp, :],
                            rhs=pt[:, o:512],
                            start=(si == 0), stop=last)
                        nc.tensor.matmul(
                            yb[:, o:512], lhsT=va[si][:, 2 * hp + 1, :],
                            rhs=pt[:, 512 + o:1024],
                            start=(si == 0), stop=last)

                    prev = None  # (pt, si): PV runs one s-tile behind S/exp
                    for si in range(nsi):
                        m = si - 4 * tj  # diagonal-band index (>=0 on diag)
                        o = max(m, 0) * P  # first valid column in this block
                        s = ps_s.tile([P, 1024], F32, tag="ps_s")
                        nc.tensor.matmul(
                            s[:, o:512], lhsT=kt[0:D, si * P:(si + 1) * P],
                            rhs=qt[0:D, tj * 512 + o:(tj + 1) * 512],
                            start=True, stop=True)
                        nc.tensor.matmul(
                            s[:, 512 + o:1024], lhsT=kt[D:P, si * P:(si + 1) * P],
                            rhs=qt[D:P, tj * 512 + o:(tj + 1) * 512],
                            start=True, stop=True)
                        pt = p_pt.tile([P, 1024], DT, tag="pt")
                        if m < 0:
                            nc.scalar.activation(pt, s, EXP, scale=0.125)
                        else:
                            # one strided call covers both heads' valid range
                            pt2 = pt.rearrange("p (h w) -> p h w", h=2)
                            s2 = s.rearrange("p (h w) -> p h w", h=2)
                            nc.scalar.activation(pt2[:, :, o:512], s2[:, :, o:512],
                                                 EXP, scale=0.125)
                            nc.gpsimd.tensor_tensor(
                                pt2[:, :, o:o + P], pt2[:, :, o:o + P],
                                tri_sb[:, None, :].to_broadcast((P, 2, P)),
                                mybir.AluOpType.mult)
                        if prev is not None:
                            pv_step(*prev, False)
                        prev = (pt, si)
                        # deficit accounting: exp minus S/PV time this step
                        nv = 512 - o
                        deficit += (2 * nv + 352) / 1.2 - (3 * nv / 2.4 + 140)
                        # near the end, hold back 2 fillers to cover the
                        # final norm chain's reciprocal-bounce latency
                        reserve = 2 if bi >= len(blocks) - 2 else 0
                        while (fillq and len(fillq) > reserve
                               and deficit >= fillq[0][0] * 0.6):
                            deficit -= pop_front()
                    pv_step(*prev, True)
                    # evacuate Y quickly: copy unnormalized Y (sums ride in
                    # row D) into one [65,1024] tile, reciprocal via direct
                    # SBUF->SBUF restriding DMAs to [128, 8] and back
                    yun = p_yun.tile([D + 1, 1024], F32, tag="yun", name="yun")
                    nc.vector.tensor_copy(yun[:, 0:512], ya)
                    nc.vector.tensor_copy(yun[:, 512:1024], yb)
                    scat = p_scat.tile([P, 8], F32, tag="scat", name="scat")
                    nc.sync.dma_start(scat, yun[D:D + 1, :])
                    scatr = p_scat.tile([P, 8], mybir.dt.float32r, tag="scatr", name="scatr")
                    with nc.allow_low_precision(reason="elementwise recip"):
                        nc.vector.reciprocal(scatr, scat)
                    rcp_row = p_rcpr.tile([1, 1024], mybir.dt.float32r, tag="rcpr", name="rcp_row")
                    nc.sync.dma_start(rcp_row[0:1, :], scatr)
                    pending.append((hp, tj, rcp_row, yun))
                    if len(pending) >= 2:
                        emit_norm(pending.pop(0))
            while pending:
                emit_norm(pending.pop(0))
                n = 2
                while fillq and n:
                    pop_front()
                    n -= 1
            while fillq:
                pop_front()

    nc.compile()
    return nc


_PROG = None


def _get_prog():
    global _PROG
    if _PROG is None:
        _PROG = build_program()
    return _PROG


_LAST_RESULT = {}


def kernel(x, W_attn, b_attn, W_proj, b_proj):
    x = np.asarray(x, np.float32)
    W_attn = np.asarray(W_attn, np.float32)
    b_attn = np.asarray(b_attn, np.float32)
    W_proj = np.asarray(W_proj, np.float32)
    b_proj = np.asarray(b_proj, np.float32)
    B = x.shape[0]
    nc = _get_prog()
    f = np.arange(P)[None, :]
    p = np.arange(P)[:, None]
    tri = (f >= p).astype(NPDT)
    cvt = lambda a: np.ascontiguousarray(a).astype(NPDT)
    in_maps = []
    for c in range(2 * B):
        b, hh = divmod(c, 2)
        sl = slice(hh * QC, hh * QC + QC)
        in_maps.append({
            "xT": cvt(x[b].T),
            "wqk": cvt(np.concatenate(
                [W_attn[:, sl], W_attn[:, C + hh * QC:C + hh * QC + QC]], axis=1)),
            "bqk": np.ascontiguousarray(np.concatenate(
                [b_attn[sl], b_attn[C + hh * QC:C + hh * QC + QC]])),
            "wv": cvt(W_attn[:, 2 * C + hh * QC:2 * C + hh * QC + QC]),
            "wp": cvt(W_proj[hh * QC:hh * QC + QC, :]),
            "trimask": tri,
            "ones": np.ones((P, P), NPDT),
        })
    res = run_bass_kernel_spmd(nc, in_maps, list(range(2 * B)), trace=TRACE)
    _LAST_RESULT["res"] = res
    # v-bias folded through softmax: y = attn + b_v, so out += b_v @ W_proj
    bias = b_proj + b_attn[2 * C:] @ W_proj
    out = np.empty((B, T, C), np.float32)
    for b in range(B):
        out[b] = (res.results[2 * b]["out"].astype(np.float32)
                  + res.results[2 * b + 1]["out"].astype(np.float32) + bias)
    return out


# revision 16
# speedup vs baseline: 1.4769x; 1.4769x over previous
"""Causal self-attention (B=4, T=2048, C=1024, H=16, D=64) on 8 TRN2 cores.

Sharding: core c handles batch b = c//2 and head-half hh = c%2 (8 heads).
Each core computes the qkv projection for its heads, causal attention, and
a partial output projection (its heads' rows of W_proj). Host sums the two
partials per batch and adds b_proj.

Schedule (single software-pipelined stream; all matmuls bf16, fp32 PSUM):
  - Attention blocks (tj, hp) run tj-DESCENDING (3,2,1,0), hp 0..3, so the
    largest exp workloads come first (when qk/v projection filler work is
    plentiful) and the tail block is tiny.
  - Prefix: k-proj for hp=0, all V projection (va tiles), q-proj(hp0,tj3),
    chasing the DMA stream (wqk-ch4 + xt quarter-columns first).
  - A global filler queue holds the remaining qk-proj units and the output
    projection tiles; between attention s-tiles the scheduler pops fillers
    by a deficit model (ScalarE exp time minus PE S/PV time per s-tile) to
    keep the PE busy while exp runs.
  - DMA instruction count is minimized (the Sync engine issues descriptors
    serially at ~600ns each): xt is one [128,8,2048] tile filled by 4
    column-quarter DMAs, wqk one DMA per 128-channel group, wv/wp one DMA
    each, and the va ones-column is written by GpSimd copies, not DMA.
  - Causal diagonal handled by narrowing S/exp/PV to the valid column
    range plus a [128,2,128] triangular mask multiply on GpSimd.
  - Normalization: row sums ride in PV output row D (ones column in va);
    one SBUF restriding DMA to a [128,8] scatter layout, DVE reciprocal,
    one bounce back to a [1,1024] row, broadcast across the 64 partitions
    with a K=1 matmul, applied in the DVE mult into ysb.
"""

from contextlib import ExitStack

import ml_dtypes
import numpy as np

import concourse.bass as bass
import concourse.tile as tile
from concourse import bacc, mybir
from concourse.bass_utils import run_bass_kernel_spmd

F32 = mybir.dt.float32
DT = mybir.dt.bfloat16
NPDT = ml_dtypes.bfloat16
EXP = mybir.ActivationFunctionType.Exp

T = 2048        # tokens per core (one batch element)
C = 1024        # embed dim
H = 8           # local heads per core
D = 64          # head dim
P = 128
CT = C // P     # 8 contraction tiles over embed dim
QC = H * D      # 512 q/k/v channels per core
TJN = T // 512  # 4 t-tiles (free dim) for attention
SIN = T // P    # 16 s-tiles

TRACE = False   # set by test.py for profiling runs


def build_program():
    nc = bacc.Bacc("TRN2", target_bir_lowering=False, debug=False)
    xT = nc.dram_tensor("xT", [C, T], DT, kind="ExternalInput").ap()
    wqk = nc.dram_tensor("wqk", [C, 2 * QC], DT, kind="ExternalInput").ap()
    bqk = nc.dram_tensor("bqk", [2 * QC], F32, kind="ExternalInput").ap()
    wv = nc.dram_tensor("wv", [C, QC], DT, kind="ExternalInput").ap()
    wp = nc.dram_tensor("wp", [QC, C], DT, kind="ExternalInput").ap()
    trimask = nc.dram_tensor("trimask", [P, P], DT, kind="ExternalInput").ap()
    ones_in = nc.dram_tensor("ones", [P, P], DT, kind="ExternalInput").ap()
    out = nc.dram_tensor("out", [T, C], DT, kind="ExternalOutput").ap()

    with tile.TileContext(nc) as tc, ExitStack() as persist:
        p_small = persist.enter_context(tc.tile_pool(name="small", bufs=1))
        p_qkt = persist.enter_context(tc.tile_pool(name="qkt", bufs=1))
        p_va = persist.enter_context(tc.tile_pool(name="va", bufs=1))
        qkt = [p_qkt.tile([P, T], DT, tag=f"qkt{i}", name=f"qkt{i}") for i in range(CT)]
        va = [p_va.tile([P, H, D + 1], DT, tag=f"va{i}", name=f"va{i}") for i in range(SIN)]

        with ExitStack() as ph:
            p_xt = ph.enter_context(tc.tile_pool(name="xt", bufs=1))
            p_wqk = ph.enter_context(tc.tile_pool(name="wqk", bufs=8))
            p_wv = ph.enter_context(tc.tile_pool(name="wv", bufs=1))
            xt = p_xt.tile([P, CT, T], DT, tag="xt", name="xt")
            wv_sb = p_wv.tile([P, CT, QC], DT, tag="wv", name="wv")

            p_ysb = ph.enter_context(tc.tile_pool(name="ysb", bufs=1))
            ysb = [p_ysb.tile([P, T], DT, tag=f"ysb{i}", name=f"ysb{i}")
                   for i in range(QC // P)]
            p_wp = ph.enter_context(tc.tile_pool(name="wp", bufs=1))
            wpt = p_wp.tile([P, QC // P, C], DT, tag="wp", name="wp")
            p_pt = ph.enter_context(tc.tile_pool(name="pt", bufs=4))
            p_scat = ph.enter_context(tc.tile_pool(name="scat", bufs=4))
            p_rcpr = ph.enter_context(tc.tile_pool(name="rcpr", bufs=5))
            p_yun = ph.enter_context(tc.tile_pool(name="yun", bufs=5))
            p_yn = ph.enter_context(tc.tile_pool(name="yn", bufs=3))
            p_rb = ph.enter_context(tc.tile_pool(name="rb", bufs=3))
            p_o = ph.enter_context(tc.tile_pool(name="o", bufs=2))
            ps_s = ph.enter_context(tc.tile_pool(name="ps_s", bufs=2, space="PSUM"))
            ps_y = ph.enter_context(tc.tile_pool(name="ps_y", bufs=2, space="PSUM"))
            # shared by v-proj, qk-proj, normalization R, and proj outputs
            ps_r = ph.enter_context(tc.tile_pool(name="ps_r", bufs=2, space="PSUM"))

            # ---------- DMA emission (defines Sync-queue order) ----------
            qk_w = [None] * (2 * QC // P)

            def dma_qk_w(ch):
                t = p_wqk.tile([P, CT, P], DT, tag="wqk", name=f"wqk{ch}")
                nc.sync.dma_start(
                    t, wqk.rearrange("(j p) c -> p j c", p=P)[:, :, ch * P:(ch + 1) * P])
                qk_w[ch] = t

            xTr = xT.rearrange("(j p) t -> p j t", p=P)
            dma_qk_w(4)                      # k-weights for hp=0 first
            # first x quarter in two halves so qkproj(4,0) starts sooner
            nc.sync.dma_start(xt[:, 0:4, 0:512], xTr[:, 0:4, 0:512])
            nc.sync.dma_start(xt[:, 4:8, 0:512], xTr[:, 4:8, 0:512])
            nc.sync.dma_start(wv_sb, wv.rearrange("(j p) c -> p j c", p=P))

            bqk_sb = p_small.tile([P, CT], F32, tag="bqk")
            nc.sync.dma_start(bqk_sb, bqk.rearrange("(j p) -> p j", p=P))
            ones_sb = p_small.tile([P, P], DT, tag="ones_sb")
            nc.sync.dma_start(ones_sb, ones_in)
            tri_sb = p_small.tile([P, P], DT, tag="tri")
            nc.sync.dma_start(tri_sb, trimask)
            onesf_sb = p_small.tile([P, D], mybir.dt.float32r, tag="onesf_sb")
            nc.vector.tensor_copy(onesf_sb, ones_sb[:, 0:D])

            for tq in range(1, 4):           # remaining x quarters
                nc.sync.dma_start(xt[:, :, tq * 512:(tq + 1) * 512],
                                  xTr[:, :, tq * 512:(tq + 1) * 512])
            dma_qk_w(0)
            for ch in (5, 1, 6, 2, 7, 3):
                dma_qk_w(ch)
            nc.sync.dma_start(wpt, wp.rearrange("(i p) c -> p i c", p=P))

            # ---------- work units ----------
            def qkproj(ch, tjc):
                wt = qk_w[ch]
                pq = ps_r.tile([P, 512], F32, tag="ps_r", name="pq")
                for j in range(CT):
                    nc.tensor.matmul(
                        pq, lhsT=wt[:, j, :],
                        rhs=xt[:, j, tjc * 512:(tjc + 1) * 512],
                        start=(j == 0), stop=(j == CT - 1))
                nc.vector.tensor_scalar_add(
                    out=qkt[ch][:, tjc * 512:(tjc + 1) * 512],
                    in0=pq, scalar1=bqk_sb[:, ch:ch + 1])

            def vproj(g):
                # v-bias omitted: softmax weights sum to 1, so b_v passes
                # through attention unchanged; host folds b_v @ W_proj into
                # the output bias instead.
                for tt in (2 * g, 2 * g + 1):
                    pv = ps_r.tile([P, QC], F32, tag="ps_r", name="pv")
                    for j in range(CT):
                        nc.tensor.matmul(
                            pv, lhsT=xt[:, j, tt * P:(tt + 1) * P],
                            rhs=wv_sb[:, j, :], start=(j == 0),
                            stop=(j == CT - 1))
                    nc.vector.tensor_copy(out=va[tt][:, :, 0:D], in_=pv)
                    nc.gpsimd.tensor_copy(out=va[tt][:, :, D:D + 1],
                                          in_=ones_sb[:, 0:H][:, :, None])

            def proj_tile(tt):
                ot = p_o.tile([P, C], DT, tag="o", name="ot")
                for co in range(C // 512):
                    po = ps_r.tile([P, 512], F32, tag="ps_r", name="po")
                    for i in range(QC // P):
                        nc.tensor.matmul(
                            po, lhsT=ysb[i][:, tt * P:(tt + 1) * P],
                            rhs=wpt[:, i, co * 512:(co + 1) * 512],
                            start=(i == 0), stop=(i == QC // P - 1))
                    nc.vector.tensor_copy(ot[:, co * 512:(co + 1) * 512], po)
                nc.sync.dma_start(out[tt * P:(tt + 1) * P, :], ot)

            def norm_batch(hp, tj, rcp_row, yun):
                ts = slice(tj * 512, (tj + 1) * 512)
                for head in (0, 1):
                    r = ps_r.tile([P, 512], F32, tag="ps_r", name="r")
                    nc.tensor.matmul(
                        r[0:D, :], lhsT=onesf_sb[0:1, :],
                        rhs=rcp_row[0:1, head * 512:(head + 1) * 512],
                        start=True, stop=True)
                    if head == 0:
                        nc.vector.tensor_mul(ysb[hp][0:D, ts],
                                             yun[0:D, 0:512], r[0:D, :])
                    else:
                        ynb = p_yn.tile([D, 512], DT, tag="yn", name="ynb")
                        nc.vector.tensor_mul(ynb, yun[0:D, 512:1024], r[0:D, :])
                        nc.sync.dma_start(ysb[hp][D:P, ts], ynb)

            # ---------- filler queue ----------
            fillq = []
            emitted = set()

            def push(cost, key, fn):
                fillq.append((cost, key, fn))

            def pop_front():
                cost, key, fn = fillq.pop(0)
                emitted.add(key)
                fn()
                return cost

            def ensure(key):
                while key not in emitted:
                    pop_front()

            QP = 1760.0   # ns per 8-MM qk-proj unit
            OP = 1760.0   # ns per out-proj row-tile (both column halves)
            for hp in range(1, 4):
                for tjc in range(TJN):
                    push(QP, ("k", hp, tjc),
                         (lambda c=4 + hp, t=tjc: qkproj(c, t)))
                push(QP, ("q", hp, 3), (lambda c=hp: qkproj(c, 3)))
            for tj in (2, 1, 0):
                for hp in range(4):
                    push(QP, ("q", hp, tj),
                         (lambda c=hp, t=tj: qkproj(c, t)))

            # ---------- prefix: hp0 k-proj + all V proj + q(0,3) ----------
            qkproj(4, 0)
            vproj(0)
            vproj(1)
            qkproj(4, 1)
            vproj(2)
            vproj(3)
            qkproj(4, 2)
            vproj(4)
            vproj(5)
            qkproj(4, 3)
            vproj(6)
            vproj(7)
            qkproj(0, 3)
            for t in range(TJN):
                emitted.add(("k", 0, t))
            emitted.add(("q", 0, 3))

            # ---------- attention blocks ----------
            pending = []
            norms_emitted = {tj: 0 for tj in range(TJN)}
            deficit = 0.0

            def emit_norm(args):
                hp, tj = args[0], args[1]
                norm_batch(*args)
                norms_emitted[tj] += 1
                if norms_emitted[tj] == 4:
                    for tt in range(4 * tj, 4 * tj + 4):
                        push(OP, ("o", tt), (lambda a=tt: proj_tile(a)))

            blocks = [(tj, hp) for tj in (3, 2, 1, 0) for hp in range(4)]
            for bi, (tj, hp) in enumerate(blocks):
                    if (tj, hp) != (3, 0):
                        ensure(("q", hp, tj))
                    if bi + 1 < len(blocks):
                        # prefetch next block's q-proj so its S matmuls
                        # don't wait on the PSUM->qkt cast latency
                        ntj, nhp = blocks[bi + 1]
                        ensure(("q", nhp, ntj))
                    qt, kt = qkt[hp], qkt[4 + hp]
                    nsi = 4 * tj + 4
                    ya = ps_y.tile([D + 1, 512], F32, tag="ps_y")
                    yb = ps_y.tile([D + 1, 512], F32, tag="ps_y")

                    def pv_step(pt, si, last):
                        o = max(si - 4 * tj, 0) * P
                        nc.tensor.matmul(
                            ya[:, o:512], lhsT=va[si][:, 2 * hp, :],
                            rhs=pt[:, o:512],
                            start=(si == 0), stop=last)
                        nc.tensor.matmul(
                            yb[:, o:512], lhsT=va[si][:, 2 * hp + 1, :],
                            rhs=pt[:, 512 + o:1024],
                            start=(si == 0), stop=last)

                    prev = None  # (pt, si): PV runs one s-tile behind S/exp
                    for si in range(nsi):
                        m = si - 4 * tj  # diagonal-band index (>=0 on diag)
                        o = max(m, 0) * P  # first valid column in this block
                        s = ps_s.tile([P, 1024], F32, tag="ps_s")
                        nc.tensor.matmul(
                            s[:, o:512], lhsT=kt[0:D, si * P:(si + 1) * P],
                            rhs=qt[0:D, tj * 512 + o:(tj + 1) * 512],
                            start=True, stop=True)
                        nc.tensor.matmul(
                            s[:, 512 + o:1024], lhsT=kt[D:P, si * P:(si + 1) * P],
                            rhs=qt[D:P, tj * 512 + o:(tj + 1) * 512],
                            start=True, stop=True)
                        pt = p_pt.tile([P, 1024], DT, tag="pt")
                        if m < 0:
                            nc.scalar.activation(pt, s, EXP, scale=0.125)
                        else:
                            # one strided call covers both heads' valid range
                            pt2 = pt.rearrange("p (h w) -> p h w", h=2)
                            s2 = s.rearrange("p (h w) -> p h w", h=2)
                            nc.scalar.activation(pt2[:, :, o:512], s2[:, :, o:512],
                                                 EXP, scale=0.125)
                            nc.gpsimd.tensor_tensor(
                                pt2[:, :, o:o + P], pt2[:, :, o:o + P],
                                tri_sb[:, None, :].to_broadcast((P, 2, P)),
                                mybir.AluOpType.mult)
                        if prev is not None:
                            pv_step(*prev, False)
                        prev = (pt, si)
                        # deficit accounting: exp minus S/PV time this step
                        nv = 512 - o
                        deficit += (2 * nv + 352) / 1.2 - (3 * nv / 2.4 + 140)
                        # near the end, hold back 2 fillers to cover the
                        # final norm chain's reciprocal-bounce latency
                        reserve = 2 if bi >= len(blocks) - 2 else 0
                        while (fillq and len(fillq) > reserve
                               and deficit >= fillq[0][0] * 0.6):
                            deficit -= pop_front()
                    pv_step(*prev, True)
                    # evacuate Y quickly: copy unnormalized Y (sums ride in
                    # row D) into one [65,1024] tile, reciprocal via direct
                    # SBUF->SBUF restriding DMAs to [128, 8] and back
                    yun = p_yun.tile([D + 1, 1024], F32, tag="yun", name="yun")
                    nc.vector.tensor_copy(yun[:, 0:512], ya)
                    nc.vector.tensor_copy(yun[:, 512:1024], yb)
                    scat = p_scat.tile([P, 8], F32, tag="scat", name="scat")
                    nc.sync.dma_start(scat, yun[D:D + 1, :])
                    scatr = p_scat.tile([P, 8], mybir.dt.float32r, tag="scatr", name="scatr")
                    with nc.allow_low_precision(reason="elementwise recip"):
                        nc.vector.reciprocal(scatr, scat)
                    rcp_row = p_rcpr.tile([1, 1024], mybir.dt.float32r, tag="rcpr", name="rcp_row")
                    nc.sync.dma_start(rcp_row[0:1, :], scatr)
                    pending.append((hp, tj, rcp_row, yun))
                    if len(pending) >= 2:
                        emit_norm(pending.pop(0))
            while pending:
                emit_norm(pending.pop(0))
                n = 2
                while fillq and n:
                    pop_front()
                    n -= 1
            while fillq:
                pop_front()

    nc.compile()
    return nc


_PROG = None


def _get_prog():
    global _PROG
    if _PROG is None:
        _PROG = build_program()
    return _PROG


_LAST_RESULT = {}


def kernel(x, W_attn, b_attn, W_proj, b_proj):
    x = np.asarray(x, np.float32)
    W_attn = np.asarray(W_attn, np.float32)
    b_attn = np.asarray(b_attn, np.float32)
    W_proj = np.asarray(W_proj, np.float32)
    b_proj = np.asarray(b_proj, np.float32)
    B = x.shape[0]
    nc = _get_prog()
    f = np.arange(P)[None, :]
    p = np.arange(P)[:, None]
    tri = (f >= p).astype(NPDT)
    cvt = lambda a: np.ascontiguousarray(a).astype(NPDT)
    in_maps = []
    for c in range(2 * B):
        b, hh = divmod(c, 2)
        sl = slice(hh * QC, hh * QC + QC)
        in_maps.append({
            "xT": cvt(x[b].T),
            "wqk": cvt(np.concatenate(
                [W_attn[:, sl], W_attn[:, C + hh * QC:C + hh * QC + QC]], axis=1)),
            "bqk": np.ascontiguousarray(np.concatenate(
                [b_attn[sl], b_attn[C + hh * QC:C + hh * QC + QC]])),
            "wv": cvt(W_attn[:, 2 * C + hh * QC:2 * C + hh * QC + QC]),
            "wp": cvt(W_proj[hh * QC:hh * QC + QC, :]),
            "trimask": tri,
            "ones": np.ones((P, P), NPDT),
        })
    res = run_bass_kernel_spmd(nc, in_maps, list(range(2 * B)), trace=TRACE)
    _LAST_RESULT["res"] = res
    # v-bias folded through softmax: y = attn + b_v, so out += b_v @ W_proj
    bias = b_proj + b_attn[2 * C:] @ W_proj
    out = np.empty((B, T, C), np.float32)
    for b in range(B):
        out[b] = (res.results[2 * b]["out"].astype(np.float32)
                  + res.results[2 * b + 1]["out"].astype(np.float32) + bias)
    return out


# revision 21
# speedup vs baseline: 1.5119x; 1.0237x over previous
"""Causal self-attention (B=4, T=2048, C=1024, H=16, D=64) on 8 TRN2 cores.

Sharding: core c handles batch b = c//2 and head-half hh = c%2 (8 heads).
Each core computes the qkv projection for its heads, causal attention, and
a partial output projection (its heads' rows of W_proj). Host sums the two
partials per batch and adds b_proj.

Schedule (single software-pipelined stream; all matmuls bf16, fp32 PSUM):
  - Attention blocks (tj, hp) run tj-DESCENDING (3,2,1,0), hp 0..3, so the
    largest exp workloads come first (when qk/v projection filler work is
    plentiful) and the tail block is tiny.
  - Prefix: k-proj for hp=0, all V projection (va tiles), q-proj(hp0,tj3),
    chasing the DMA stream (wqk-ch4 + xt quarter-columns first).
  - A global filler queue holds the remaining qk-proj units and the output
    projection tiles; between attention s-tiles the scheduler pops fillers
    by a deficit model (ScalarE exp time minus PE S/PV time per s-tile) to
    keep the PE busy while exp runs.
  - DMA instruction count is minimized (the Sync engine issues descriptors
    serially at ~600ns each): xt is one [128,8,2048] tile filled by 4
    column-quarter DMAs, wqk one DMA per 128-channel group, wv/wp one DMA
    each, and the va ones-column is written by GpSimd copies, not DMA.
  - Causal diagonal handled by narrowing S/exp/PV to the valid column
    range plus a [128,2,128] triangular mask multiply on GpSimd.
  - Normalization: row sums ride in PV output row D (ones column in va);
    one SBUF restriding DMA to a [128,8] scatter layout, DVE reciprocal,
    one bounce back to a [1,1024] row, broadcast across the 64 partitions
    with a K=1 matmul, applied in the DVE mult into ysb.
"""

from contextlib import ExitStack

import ml_dtypes
import numpy as np

import concourse.bass as bass
import concourse.tile as tile
from concourse import bacc, mybir
from concourse.bass_utils import run_bass_kernel_spmd

F32 = mybir.dt.float32
DT = mybir.dt.bfloat16
NPDT = ml_dtypes.bfloat16
EXP = mybir.ActivationFunctionType.Exp

T = 2048        # tokens per core (one batch element)
C = 1024        # embed dim
H = 8           # local heads per core
D = 64          # head dim
P = 128
CT = C // P     # 8 contraction tiles over embed dim
QC = H * D      # 512 q/k/v channels per core
TJN = T // 512  # 4 t-tiles (free dim) for attention
SIN = T // P    # 16 s-tiles

TRACE = False   # set by test.py for profiling runs


def build_program():
    nc = bacc.Bacc("TRN2", target_bir_lowering=False, debug=False)
    xT = nc.dram_tensor("xT", [C, T], DT, kind="ExternalInput").ap()
    wqk = nc.dram_tensor("wqk", [C, 2 * QC], DT, kind="ExternalInput").ap()
    bqk = nc.dram_tensor("bqk", [2 * QC], F32, kind="ExternalInput").ap()
    wv = nc.dram_tensor("wv", [C, QC], DT, kind="ExternalInput").ap()
    wp = nc.dram_tensor("wp", [QC, C], DT, kind="ExternalInput").ap()
    trimask = nc.dram_tensor("trimask", [P, P], DT, kind="ExternalInput").ap()
    ones_in = nc.dram_tensor("ones", [P, P], DT, kind="ExternalInput").ap()
    out = nc.dram_tensor("out", [T, C], DT, kind="ExternalOutput").ap()

    with tile.TileContext(nc) as tc, ExitStack() as persist:
        p_small = persist.enter_context(tc.tile_pool(name="small", bufs=1))
        p_qkt = persist.enter_context(tc.tile_pool(name="qkt", bufs=1))
        p_va = persist.enter_context(tc.tile_pool(name="va", bufs=1))
        qkt = [p_qkt.tile([P, T], DT, tag=f"qkt{i}", name=f"qkt{i}") for i in range(CT)]
        va = [p_va.tile([P, H, D + 1], DT, tag=f"va{i}", name=f"va{i}") for i in range(SIN)]

        with ExitStack() as ph:
            p_xt = ph.enter_context(tc.tile_pool(name="xt", bufs=1))
            p_wqk = ph.enter_context(tc.tile_pool(name="wqk", bufs=8))
            p_wv = ph.enter_context(tc.tile_pool(name="wv", bufs=1))
            xt = p_xt.tile([P, CT, T], DT, tag="xt", name="xt")
            wv_sb = p_wv.tile([P, CT, QC], DT, tag="wv", name="wv")

            p_ysb = ph.enter_context(tc.tile_pool(name="ysb", bufs=1))
            ysb = [p_ysb.tile([P, T], DT, tag=f"ysb{i}", name=f"ysb{i}")
                   for i in range(QC // P)]
            p_wp = ph.enter_context(tc.tile_pool(name="wp", bufs=1))
            wpt = p_wp.tile([P, QC // P, C], DT, tag="wp", name="wp")
            p_pt = ph.enter_context(tc.tile_pool(name="pt", bufs=5))
            p_scat = ph.enter_context(tc.tile_pool(name="scat", bufs=4))
            p_rcpr = ph.enter_context(tc.tile_pool(name="rcpr", bufs=5))
            p_yun = ph.enter_context(tc.tile_pool(name="yun", bufs=5))
            p_yn = ph.enter_context(tc.tile_pool(name="yn", bufs=3))
            p_rb = ph.enter_context(tc.tile_pool(name="rb", bufs=3))
            p_o = ph.enter_context(tc.tile_pool(name="o", bufs=2))
            ps_s = ph.enter_context(tc.tile_pool(name="ps_s", bufs=2, space="PSUM"))
            ps_y = ph.enter_context(tc.tile_pool(name="ps_y", bufs=2, space="PSUM"))
            # shared by v-proj, qk-proj, normalization R, and proj outputs
            ps_r = ph.enter_context(tc.tile_pool(name="ps_r", bufs=2, space="PSUM"))

            # ---------- DMA emission (defines Sync-queue order) ----------
            qk_w = [None] * (2 * QC // P)

            def dma_qk_w(ch):
                t = p_wqk.tile([P, CT, P], DT, tag="wqk", name=f"wqk{ch}")
                nc.sync.dma_start(
                    t, wqk.rearrange("(j p) c -> p j c", p=P)[:, :, ch * P:(ch + 1) * P])
                qk_w[ch] = t

            xTr = xT.rearrange("(j p) t -> p j t", p=P)
            wqkr = wqk.rearrange("(j p) c -> p j c", p=P)
            # interleave the first weight/x halves so qkproj(4,0) starts
            # as early as possible
            w4 = p_wqk.tile([P, CT, P], DT, tag="wqk", name="wqk4")
            qk_w[4] = w4
            nc.sync.dma_start(w4[:, 0:4, :], wqkr[:, 0:4, 4 * P:5 * P])
            nc.sync.dma_start(xt[:, 0:4, 0:512], xTr[:, 0:4, 0:512])
            nc.sync.dma_start(w4[:, 4:8, :], wqkr[:, 4:8, 4 * P:5 * P])
            nc.sync.dma_start(xt[:, 4:8, 0:512], xTr[:, 4:8, 0:512])
            nc.sync.dma_start(wv_sb, wv.rearrange("(j p) c -> p j c", p=P))

            bqk_sb = p_small.tile([P, CT], F32, tag="bqk")
            nc.sync.dma_start(bqk_sb, bqk.rearrange("(j p) -> p j", p=P))
            ones_sb = p_small.tile([P, P], DT, tag="ones_sb")
            nc.sync.dma_start(ones_sb, ones_in)
            tri_sb = p_small.tile([P, P], DT, tag="tri")
            nc.sync.dma_start(tri_sb, trimask)
            onesf_sb = p_small.tile([P, D], mybir.dt.float32r, tag="onesf_sb")
            nc.vector.tensor_copy(onesf_sb, ones_sb[:, 0:D])

            for tq in range(1, 4):           # remaining x quarters
                nc.sync.dma_start(xt[:, :, tq * 512:(tq + 1) * 512],
                                  xTr[:, :, tq * 512:(tq + 1) * 512])
            dma_qk_w(0)
            for ch in (5, 1, 6, 2, 7, 3):
                dma_qk_w(ch)
            nc.sync.dma_start(wpt, wp.rearrange("(i p) c -> p i c", p=P))

            # ---------- work units ----------
            def qkproj(ch, tjc):
                wt = qk_w[ch]
                pq = ps_r.tile([P, 512], F32, tag="ps_r", name="pq")
                for j in range(CT):
                    nc.tensor.matmul(
                        pq, lhsT=wt[:, j, :],
                        rhs=xt[:, j, tjc * 512:(tjc + 1) * 512],
                        start=(j == 0), stop=(j == CT - 1))
                nc.vector.tensor_scalar_add(
                    out=qkt[ch][:, tjc * 512:(tjc + 1) * 512],
                    in0=pq, scalar1=bqk_sb[:, ch:ch + 1])

            def vproj(g):
                # v-bias omitted: softmax weights sum to 1, so b_v passes
                # through attention unchanged; host folds b_v @ W_proj into
                # the output bias instead.
                for tt in (2 * g, 2 * g + 1):
                    pv = ps_r.tile([P, QC], F32, tag="ps_r", name="pv")
                    for j in range(CT):
                        nc.tensor.matmul(
                            pv, lhsT=xt[:, j, tt * P:(tt + 1) * P],
                            rhs=wv_sb[:, j, :], start=(j == 0),
                            stop=(j == CT - 1))
                    nc.vector.tensor_copy(out=va[tt][:, :, 0:D], in_=pv)
                    nc.gpsimd.tensor_copy(out=va[tt][:, :, D:D + 1],
                                          in_=ones_sb[:, 0:H][:, :, None])

            def proj_tile(tt):
                ot = p_o.tile([P, C], DT, tag="o", name="ot")
                for co in range(C // 512):
                    po = ps_r.tile([P, 512], F32, tag="ps_r", name="po")
                    for i in range(QC // P):
                        nc.tensor.matmul(
                            po, lhsT=ysb[i][:, tt * P:(tt + 1) * P],
                            rhs=wpt[:, i, co * 512:(co + 1) * 512],
                            start=(i == 0), stop=(i == QC // P - 1))
                    nc.vector.tensor_copy(ot[:, co * 512:(co + 1) * 512], po)
                nc.sync.dma_start(out[tt * P:(tt + 1) * P, :], ot)

            def norm_batch(hp, tj, rcp_row, yun):
                ts = slice(tj * 512, (tj + 1) * 512)
                for head in (0, 1):
                    r = ps_r.tile([P, 512], F32, tag="ps_r", name="r")
                    nc.tensor.matmul(
                        r[0:D, :], lhsT=onesf_sb[0:1, :],
                        rhs=rcp_row[0:1, head * 512:(head + 1) * 512],
                        start=True, stop=True)
                    if head == 0:
                        nc.vector.tensor_mul(ysb[hp][0:D, ts],
                                             yun[0:D, 0:512], r[0:D, :])
                    else:
                        ynb = p_yn.tile([D, 512], DT, tag="yn", name="ynb")
                        nc.vector.tensor_mul(ynb, yun[0:D, 512:1024], r[0:D, :])
                        nc.sync.dma_start(ysb[hp][D:P, ts], ynb)

            # ---------- filler queue ----------
            fillq = []
            emitted = set()

            def push(cost, key, fn):
                fillq.append((cost, key, fn))

            def pop_front():
                cost, key, fn = fillq.pop(0)
                emitted.add(key)
                fn()
                return cost

            def ensure(key):
                while key not in emitted:
                    pop_front()

            QP = 1760.0   # ns per 8-MM qk-proj unit
            OP = 1760.0   # ns per out-proj row-tile (both column halves)
            for hp in range(1, 4):
                for tjc in range(TJN):
                    push(QP, ("k", hp, tjc),
                         (lambda c=4 + hp, t=tjc: qkproj(c, t)))
                push(QP, ("q", hp, 3), (lambda c=hp: qkproj(c, 3)))
            for tj in (2, 1, 0):
                for hp in range(4):
                    push(QP, ("q", hp, tj),
                         (lambda c=hp, t=tj: qkproj(c, t)))

            # ---------- prefix: hp0 k-proj + all V proj + q(0,3) ----------
            qkproj(4, 0)
            vproj(0)
            vproj(1)
            qkproj(4, 1)
            vproj(2)
            vproj(3)
            qkproj(4, 2)
            vproj(4)
            vproj(5)
            qkproj(4, 3)
            vproj(6)
            vproj(7)
            qkproj(0, 3)
            for t in range(TJN):
                emitted.add(("k", 0, t))
            emitted.add(("q", 0, 3))

            # ---------- attention blocks ----------
            pending = []
            norms_emitted = {tj: 0 for tj in range(TJN)}
            deficit = 0.0

            def emit_norm(args):
                hp, tj = args[0], args[1]
                norm_batch(*args)
                norms_emitted[tj] += 1
                if norms_emitted[tj] == 4:
                    for tt in range(4 * tj, 4 * tj + 4):
                        push(OP, ("o", tt), (lambda a=tt: proj_tile(a)))

            blocks = [(tj, hp) for tj in (3, 2, 1, 0) for hp in range(4)]
            for bi, (tj, hp) in enumerate(blocks):
                    if (tj, hp) != (3, 0):
                        ensure(("q", hp, tj))
                    if bi + 1 < len(blocks):
                        # prefetch next block's q-proj so its S matmuls
                        # don't wait on the PSUM->qkt cast latency
                        ntj, nhp = blocks[bi + 1]
                        ensure(("q", nhp, ntj))
                    qt, kt = qkt[hp], qkt[4 + hp]
                    nsi = 4 * tj + 4
                    ya = ps_y.tile([D + 1, 512], F32, tag="ps_y")
                    yb = ps_y.tile([D + 1, 512], F32, tag="ps_y")

                    def pv_step(pt, si, last):
                        o = max(si - 4 * tj, 0) * P
                        nc.tensor.matmul(
                            ya[:, o:512], lhsT=va[si][:, 2 * hp, :],
                            rhs=pt[:, o:512],
                            start=(si == 0), stop=last)
                        nc.tensor.matmul(
                            yb[:, o:512], lhsT=va[si][:, 2 * hp + 1, :],
                            rhs=pt[:, 512 + o:1024],
                            start=(si == 0), stop=last)

                    pvq = []  # (pt, si): PV runs two s-tiles behind S/exp
                    for si in range(nsi):
                        m = si - 4 * tj  # diagonal-band index (>=0 on diag)
                        o = max(m, 0) * P  # first valid column in this block
                        s = ps_s.tile([P, 1024], F32, tag="ps_s")
                        nc.tensor.matmul(
                            s[:, o:512], lhsT=kt[0:D, si * P:(si + 1) * P],
                            rhs=qt[0:D, tj * 512 + o:(tj + 1) * 512],
                            start=True, stop=True)
                        nc.tensor.matmul(
                            s[:, 512 + o:1024], lhsT=kt[D:P, si * P:(si + 1) * P],
                            rhs=qt[D:P, tj * 512 + o:(tj + 1) * 512],
                            start=True, stop=True)
                        pt = p_pt.tile([P, 1024], DT, tag="pt")
                        if m < 0:
                            nc.scalar.activation(pt, s, EXP, scale=0.125)
                        else:
                            # one strided call covers both heads' valid range
                            pt2 = pt.rearrange("p (h w) -> p h w", h=2)
                            s2 = s.rearrange("p (h w) -> p h w", h=2)
                            nc.scalar.activation(pt2[:, :, o:512], s2[:, :, o:512],
                                                 EXP, scale=0.125)
                            nc.gpsimd.tensor_tensor(
                                pt2[:, :, o:o + P], pt2[:, :, o:o + P],
                                tri_sb[:, None, :].to_broadcast((P, 2, P)),
                                mybir.AluOpType.mult)
                        pvq.append((pt, si))
                        if len(pvq) > 2:
                            pv_step(*pvq.pop(0), False)
                        # deficit accounting: exp minus S/PV time this step
                        nv = 512 - o
                        deficit += (2 * nv + 352) / 1.2 - (3 * nv / 2.4 + 140)
                        # near the end, hold back 2 fillers to cover the
                        # final norm chain's reciprocal-bounce latency
                        reserve = 2 if bi >= len(blocks) - 2 else 0
                        while (fillq and len(fillq) > reserve
                               and deficit >= fillq[0][0] * 0.6):
                            deficit -= pop_front()
                    while pvq:
                        pv_step(*pvq.pop(0), len(pvq) == 0)
                    # evacuate Y quickly: copy unnormalized Y (sums ride in
                    # row D) into one [65,1024] tile, reciprocal via direct
                    # SBUF->SBUF restriding DMAs to [128, 8] and back
                    yun = p_yun.tile([D + 1, 1024], F32, tag="yun", name="yun")
                    nc.vector.tensor_copy(yun[:, 0:512], ya)
                    nc.vector.tensor_copy(yun[:, 512:1024], yb)
                    scat = p_scat.tile([P, 8], F32, tag="scat", name="scat")
                    nc.sync.dma_start(scat, yun[D:D + 1, :])
                    scatr = p_scat.tile([P, 8], mybir.dt.float32r, tag="scatr", name="scatr")
                    with nc.allow_low_precision(reason="elementwise recip"):
                        nc.vector.reciprocal(scatr, scat)
                    rcp_row = p_rcpr.tile([1, 1024], mybir.dt.float32r, tag="rcpr", name="rcp_row")
                    nc.sync.dma_start(rcp_row[0:1, :], scatr)
                    pending.append((hp, tj, rcp_row, yun))
                    if len(pending) >= 2:
                        emit_norm(pending.pop(0))
            while pending:
                emit_norm(pending.pop(0))
                n = 2
                while fillq and n:
                    pop_front()
                    n -= 1
            while fillq:
                pop_front()

    nc.compile()
    return nc


_PROG = None


def _get_prog():
    global _PROG
    if _PROG is None:
        _PROG = build_program()
    return _PROG


_LAST_RESULT = {}


def kernel(x, W_attn, b_attn, W_proj, b_proj):
    x = np.asarray(x, np.float32)
    W_attn = np.asarray(W_attn, np.float32)
    b_attn = np.asarray(b_attn, np.float32)
    W_proj = np.asarray(W_proj, np.float32)
    b_proj = np.asarray(b_proj, np.float32)
    B = x.shape[0]
    nc = _get_prog()
    f = np.arange(P)[None, :]
    p = np.arange(P)[:, None]
    tri = (f >= p).astype(NPDT)
    cvt = lambda a: np.ascontiguousarray(a).astype(NPDT)
    in_maps = []
    for c in range(2 * B):
        b, hh = divmod(c, 2)
        sl = slice(hh * QC, hh * QC + QC)
        in_maps.append({
            "xT": cvt(x[b].T),
            "wqk": cvt(np.concatenate(
                [W_attn[:, sl], W_attn[:, C + hh * QC:C + hh * QC + QC]], axis=1)),
            "bqk": np.ascontiguousarray(np.concatenate(
                [b_attn[sl], b_attn[C + hh * QC:C + hh * QC + QC]])),
            "wv": cvt(W_attn[:, 2 * C + hh * QC:2 * C + hh * QC + QC]),
            "wp": cvt(W_proj[hh * QC:hh * QC + QC, :]),
            "trimask": tri,
            "ones": np.ones((P, P), NPDT),
        })
    res = run_bass_kernel_spmd(nc, in_maps, list(range(2 * B)), trace=TRACE)
    _LAST_RESULT["res"] = res
    # v-bias folded through softmax: y = attn + b_v, so out += b_v @ W_proj
    bias = b_proj + b_attn[2 * C:] @ W_proj
    out = np.empty((B, T, C), np.float32)
    for b in range(B):
        out[b] = (res.results[2 * b]["out"].astype(np.float32)
                  + res.results[2 * b + 1]["out"].astype(np.float32) + bias)
    return out
